# revision 22
# baseline (speedup 1.0000x reference)
"""LSTM layer kernel for Trainium2 (8 NeuronCores, Bass/Tile).

Reference computation (fp32):
    z = concat([x, h], axis=1)                 # [B, IN+OUT]
    f = sigmoid(z @ w_f + b_f)
    i = sigmoid(z @ w_i + b_i)
    g = tanh   (z @ w_c + b_c)
    o = sigmoid(z @ w_o + b_o)
    c_new = c * f + i * g
    h_new = tanh(c_new) * o                    # [B, OUT]

Shapes: B=4096, IN=OUT=1024, K=IN+OUT=2048.

Sharding (8 cores): 2-D grid, 4 batch-groups x 2 output-column-groups.
Core (i, j) computes h_new[i*1024:(i+1)*1024, j*512:(j+1)*512].
Per-core HBM traffic (bf16 matmul operands): 4 MiB zT + 8 MiB weights
+ 2 MiB cT + 2 MiB out = 16 MiB.  No collectives.

Device layout (weight-stationary): contraction dim k and out-channel dim
o sit on SBUF partitions:
    zT [2048, 1024] bf16 (k, b)            - moving operand
    w  [4, 2048, 4, 128] bf16 (oc, k, gate, p) - stationary operand,
        oc-major so any ko-slice of one oc is HBM-contiguous
    psum[o, b] = w_chunk.T @ z_chunk; per-o bias is a per-partition ACT
    bias, so sigmoid/tanh drain PSUM directly (ACT reads PSUM fast; a
    DVE drain measures ~10x slower - measured, do not "fix" this).
Host pre-transposes/casts shards, transposes h_newT shards back.

bf16 operands: 1 cycle/row at N=512 (the f32r rate), but bf16 weights
get a split InstLdweights that can be DEDUPED: bass emits one ldweights
per matmul; _dedupe_ldweights removes reloads of the already-loaded
stationary tile (validated bit-exact on HW), so each weight chunk loads
once per oc and serves the two batch-half matmuls.  bf16 quantization
of z and w costs 1.9e-3 rel err vs fp32 reference (gate 2e-2).

Loop structure (per oc): gate-outer, ko-inner, batch-half pairs - gate
g's 16-ko accumulation (32 MMs, ~7us) overlaps the previous gate's ACT
drains, so PSUM banks always have slack (structures where all 8 banks
finish simultaneously stall 10-25us at pass boundaries - measured).
tanh(c*f+i*g) is emitted right after gate c so only the o-gate ACT,
one DVE mul and the out-DMA trail the last matmul.
"""

import numpy as np

import concourse.bass as bass
import concourse.tile as tile
from concourse import bacc
from concourse import mybir
from concourse.bass_utils import run_bass_kernel_spmd

P = 128
B_FULL, IN, OUT = 4096, 1024, 1024
K = IN + OUT                 # 2048 contraction
RB, RO = 4, 2                # batch-shards x out-col-shards = 8 cores
B_L = B_FULL // RB           # 1024 batch rows per core
O_L = OUT // RO              # 512 out cols per core
KO = K // P                  # 16 k-chunks
OC = O_L // P                # 4 out chunks per core
NG = 4                       # gates
NT = 512                     # moving free dim per matmul
NB = B_L // NT               # 2 batch tiles

F32 = mybir.dt.float32
BF16 = mybir.dt.bfloat16
GATES = ("f", "i", "c", "o")

# exec time of the most recent traced run (ns); set by _run when trace=True
last_exec_time_ns = None

# In the loop_r timing variant, emit this many full kernel bodies per
# For_i iteration.  tc.For_i inserts a full cross-engine drain+barrier
# reset block every iteration (~10us) which also kills cross-execution
# DMA/compute overlap; unrolling two bodies per iteration lets body k+1's
# input stream overlap body k's compute (the pools are double-buffered)
# and amortizes the reset, so the slope measures steady-state
# back-to-back execution time.  test.py divides the slope by this.
LOOP_UNROLL = 4

_NC_CACHE = {}


def _dedupe_ldweights(nc):
    """Remove InstLdweights that reload the stationary operand already in
    the PE array (same access pattern as the previous ldweights, nothing
    else touching the PE weight array in between, no sync side effects).
    bass emits one ldweights per matmul; consecutive matmuls sharing lhsT
    only need the first.  Validated bit-exact on TRN2 hardware."""
    removed = 0
    for blk in nc.m.functions[0].blocks:
        prev_key = None
        to_remove = []
        for i in list(blk.instructions):
            tn = type(i).__name__
            if tn == "InstLdweights":
                ap = i.ins[0]
                key = (ap.memref, ap.offset, str(ap.ap))
                si = i.sync_info
                has_sync = si is not None and (
                    len(si.on_wait) > 0 or len(si.on_update) > 0
                )
                if key == prev_key and not has_sync:
                    to_remove.append(i)
                else:
                    prev_key = key
            elif tn in ("InstMatmult", "InstActivation", "InstTensorTensor",
                        "InstTensorScalar", "InstTensorCopy", "InstDMACopy",
                        "InstEventSemaphore", "InstLoadActFuncSet",
                        "InstMemset", "InstIota"):
                pass  # these don't modify the PE weight array
            else:
                prev_key = None
        for i in to_remove:
            blk.instructions.remove(i)
        removed += len(to_remove)
    return removed


def _build_nc(loop_r=None):
    # loop_r: timing-only variant that repeats the whole body in a hardware
    # For_i loop (see test.py hw_loop_slope).
    nc = bacc.Bacc()

    zT = nc.dram_tensor("zT", [K, B_L], BF16, kind="ExternalInput")
    cT = nc.dram_tensor("cT", [O_L, B_L], F32, kind="ExternalInput")
    # oc-major gate-fused weights: [oc, k, gate, p], o_local = oc*128 + p
    wA = nc.dram_tensor("wA", [OC, K, NG, P], BF16, kind="ExternalInput")
    # gate-fused biases: [p, oc, gate]
    bA = nc.dram_tensor("bA", [P, OC, NG], F32, kind="ExternalInput")
    hT = nc.dram_tensor("hT", [O_L, B_L], F32, kind="ExternalOutput")

    zT_t = zT[:, :].rearrange("(ko kp) b -> kp ko b", kp=P)    # [128,16,1024]
    cT_t = cT[:, :].rearrange("(oc p) b -> p oc b", p=P)       # [128,4,1024]
    hT_t = hT[:, :].rearrange("(oc p) b -> p oc b", p=P)
    wA_t = wA[:, :, :, :].rearrange(
        "oc (ko kp) g p -> kp oc ko (g p)", kp=P
    )                                                          # [128,4,16,512]

    sig = mybir.ActivationFunctionType.Sigmoid
    tanh = mybir.ActivationFunctionType.Tanh

    import contextlib

    with tile.TileContext(nc) as tc:
        with (
            tc.For_i(0, loop_r, 1) if loop_r else contextlib.nullcontext(),
            tc.tile_pool(name="zpool", bufs=2) as zpool,
            tc.tile_pool(name="cpool", bufs=2) as cpool,
            tc.tile_pool(name="bpool", bufs=1) as bpool,
            tc.tile_pool(name="wpool", bufs=3) as wpool,
            tc.tile_pool(name="gates", bufs=2) as gpool,
            tc.tile_pool(name="temps", bufs=2) as tpool,
            tc.tile_pool(name="psum", bufs=1, space="PSUM") as psum_pool,
        ):
          for _rep in range(LOOP_UNROLL if loop_r else 1):
            z_sb = zpool.tile([P, KO, B_L], BF16, tag="z", name="z_sb")
            w_tiles = [
                wpool.tile([P, KO, NG * P], BF16, tag="w", name=f"w_oc{oc}")
                for oc in range(OC)
            ]
            c_tiles = [
                cpool.tile([P, B_L], F32, tag="c", name=f"c_oc{oc}")
                for oc in range(OC)
            ]
            b_sb = bpool.tile([P, OC, NG], F32, tag="b", name="b_sb")

            # DMA schedule.  Sync-ring FIFO = arrival order: z and the first
            # oc's weights stream together in 2-ko units (256 KiB each, both
            # HBM-contiguous) so the first gate pass starts ~2us in; later
            # weight/c tiles follow as whole contiguous blocks well before
            # the PE needs them.  Bias + c0 ride the scalar ring.
            nc.scalar.dma_start(b_sb[:, :, :], bA[:, :, :])
            nc.scalar.dma_start(c_tiles[0][:, :], cT_t[:, 0, :])
            KU = 2                                             # ko per unit
            for ku in range(KO // KU):
                ks = slice(ku * KU, (ku + 1) * KU)
                if ku == 0:
                    # 1-ko pieces, nb0 z half + w0 first, so the first
                    # matmul (gate f, nb0) launches off 256 KiB of arrivals
                    # (single-shot startup; amortized away in the loop)
                    k1s = slice(0, 1)
                    nc.sync.dma_start(z_sb[:, k1s, 0:NT], zT_t[:, k1s, 0:NT])
                    nc.sync.dma_start(w_tiles[0][:, k1s, :],
                                      wA_t[:, 0, k1s, :])
                    nc.sync.dma_start(z_sb[:, k1s, NT:B_L],
                                      zT_t[:, k1s, NT:B_L])
                    k1s = slice(1, 2)
                    nc.sync.dma_start(z_sb[:, k1s, :], zT_t[:, k1s, :])
                    nc.sync.dma_start(w_tiles[0][:, k1s, :],
                                      wA_t[:, 0, k1s, :])
                    continue
                nc.sync.dma_start(z_sb[:, ks, :], zT_t[:, ks, :])
                nc.sync.dma_start(w_tiles[0][:, ks, :], wA_t[:, 0, ks, :])
            nc.sync.dma_start(w_tiles[1][:, :, :], wA_t[:, 1, :, :])
            nc.sync.dma_start(c_tiles[1][:, :], cT_t[:, 1, :])
            nc.sync.dma_start(w_tiles[2][:, :, :], wA_t[:, 2, :, :])
            nc.sync.dma_start(c_tiles[2][:, :], cT_t[:, 2, :])
            nc.sync.dma_start(w_tiles[3][:, :, :], wA_t[:, 3, :, :])
            nc.sync.dma_start(c_tiles[3][:, :], cT_t[:, 3, :])

            for oc in range(OC):
                w_sb = w_tiles[oc]
                c_sb = c_tiles[oc]

                gate_sb = {}
                cf_sb = {}
                ps_all = {
                    g: [
                        psum_pool.tile([P, NT], F32, tag=f"ps_{g}{nb}",
                                       name=f"ps_{g}{nb}")
                        for nb in range(NB)
                    ]
                    for g in GATES
                }
                if oc == 0 and _rep == 0:
                    # ko-outer so the PE consumes each arriving z chunk for
                    # all 8 matmuls (1.7us) - slower than the DMA delivers
                    # (~1.1us) - instead of racing ahead per gate and
                    # stalling ~10us on the z stream (gate-outer consumes a
                    # chunk in 0.43us).  Uses all 8 psum banks; the oc1
                    # boundary stall is only the first ACT drains (~0.5us).
                    for ko in range(KO):
                        for gi, g in enumerate(GATES):
                            for nb in range(NB):
                                nc.tensor.matmul(
                                    ps_all[g][nb][:, :],
                                    lhsT=w_sb[:, ko, gi * P:(gi + 1) * P],
                                    rhs=z_sb[:, ko, nb * NT:(nb + 1) * NT],
                                    start=(ko == 0),
                                    stop=(ko == KO - 1),
                                )
                for gi, g in enumerate(GATES):
                    ps = ps_all[g]
                    if not (oc == 0 and _rep == 0):
                        for ko in range(KO):
                            for nb in range(NB):
                                # the two batch halves share the stationary
                                # w chunk; dedupe leaves one ldweights per ko
                                nc.tensor.matmul(
                                    ps[nb][:, :],
                                    lhsT=w_sb[:, ko, gi * P:(gi + 1) * P],
                                    rhs=z_sb[:, ko, nb * NT:(nb + 1) * NT],
                                    start=(ko == 0),
                                    stop=(ko == KO - 1),
                                )
                    func = tanh if g == "c" else sig
                    for nb in range(NB):
                        gt = gpool.tile(
                            [P, NT], F32, tag=f"gate_{g}_{nb}",
                            name=f"gate_{g}_{nb}",
                        )
                        nc.scalar.activation(
                            gt[:, :], ps[nb][:, :], func,
                            bias=b_sb[:, oc, gi:gi + 1],
                        )
                        gate_sb[(g, nb)] = gt

                    if g == "c":
                        # tanh(c*f + i*g) does not depend on gate o - emit it
                        # now so only mul+DMA remain after the last matmul
                        for nb in range(NB):
                            bsl = slice(nb * NT, (nb + 1) * NT)
                            cf = tpool.tile([P, NT], F32, tag=f"cf{nb}",
                                            name=f"cf_{nb}")
                            nc.vector.tensor_mul(
                                cf[:, :], c_sb[:, bsl],
                                gate_sb[("f", nb)][:, :]
                            )
                            ig = tpool.tile([P, NT], F32, tag=f"ig{nb}",
                                            name=f"ig{nb}")
                            nc.vector.tensor_mul(
                                ig[:, :], gate_sb[("i", nb)][:, :],
                                gate_sb[("c", nb)][:, :],
                            )
                            nc.vector.tensor_add(cf[:, :], cf[:, :], ig[:, :])
                            nc.scalar.activation(cf[:, :], cf[:, :], tanh)
                            cf_sb[nb] = cf

                for nb in range(NB):
                    bsl = slice(nb * NT, (nb + 1) * NT)
                    cf = cf_sb[nb]
                    if oc == OC - 1:
                        # half-width pieces pipeline the o-mul with the
                        # out-DMA so less trails the final matmul
                        for hh in range(2):
                            hs = slice(hh * (NT // 2), (hh + 1) * (NT // 2))
                            ob = slice(nb * NT + hh * (NT // 2),
                                       nb * NT + (hh + 1) * (NT // 2))
                            nc.vector.tensor_mul(
                                cf[:, hs], cf[:, hs],
                                gate_sb[("o", nb)][:, hs]
                            )
                            nc.gpsimd.dma_start(hT_t[:, oc, ob], cf[:, hs])
                        continue
                    nc.vector.tensor_mul(
                        cf[:, :], cf[:, :], gate_sb[("o", nb)][:, :]
                    )
                    nc.gpsimd.dma_start(hT_t[:, oc, bsl], cf[:, :])

    _dedupe_ldweights(nc)
    # run the Bacc pass pipeline (alloc_regs, wait-splitting, ...);
    # run_bass_via_pjrt does not finalize on our behalf
    nc.finalize()
    return nc


def _get_nc():
    if "nc" not in _NC_CACHE:
        _NC_CACHE["nc"] = _build_nc()
    return _NC_CACHE["nc"]


def _shard_inputs(x, h, c, w_f, b_f, w_i, b_i, w_c, b_c, w_o, b_o):
    import ml_dtypes

    ws = {"f": w_f, "i": w_i, "c": w_c, "o": w_o}
    bz = {"f": b_f, "i": b_i, "c": b_c, "o": b_o}
    f32 = np.float32
    bf16 = ml_dtypes.bfloat16

    # per-out-group fused weight/bias shards (shared by the 4 batch groups)
    # wA[oc, k, g, p] = w_g[k, j*O_L + oc*P + p]
    wA_sh = {}
    bA_sh = {}
    for j in range(RO):
        cols = slice(j * O_L, (j + 1) * O_L)
        wj = np.stack(
            [np.asarray(ws[g][:, cols], dtype=f32).reshape(K, OC, P)
             for g in GATES],
            axis=2,
        )                                       # [K, OC, NG, P]
        wA_sh[j] = np.ascontiguousarray(
            wj.transpose(1, 0, 2, 3)
        ).astype(bf16)                          # [OC, K, NG, P]
        bA_sh[j] = np.ascontiguousarray(
            np.stack(
                [np.asarray(bz[g], dtype=f32).reshape(-1)[cols].reshape(OC, P).T
                 for g in GATES],
                axis=2,
            )
        )
    in_maps = []
    for i in range(RB):
        rows = slice(i * B_L, (i + 1) * B_L)
        zT = np.ascontiguousarray(
            np.concatenate([x[rows], h[rows]], axis=1).T.astype(bf16)
        )
        for j in range(RO):
            cT = np.ascontiguousarray(
                c[rows, j * O_L:(j + 1) * O_L].T, dtype=f32
            )
            in_maps.append(
                {"zT": zT, "cT": cT, "wA": wA_sh[j], "bA": bA_sh[j]}
            )
    return in_maps


def _run(in_maps, trace=False, trace_cores=None):
    global last_exec_time_ns
    nc = _get_nc()
    res = run_bass_kernel_spmd(
        nc, in_maps, list(range(RB * RO)),
        trace=trace, trace_cores=trace_cores,
    )
    if trace:
        last_exec_time_ns = res.exec_time_ns
    return res.results


def kernel(x, h, c, w_f, b_f, w_i, b_i, w_c, b_c, w_o, b_o):
    in_maps = _shard_inputs(
        x, h, c, w_f, b_f, w_i, b_i, w_c, b_c, w_o, b_o
    )
    results = _run(in_maps)
    out = np.empty((B_FULL, OUT), np.float32)
    for i in range(RB):
        for j in range(RO):
            shard = results[i * RO + j]["hT"]  # [O_L, B_L]
            out[i * B_L:(i + 1) * B_L, j * O_L:(j + 1) * O_L] = shard.T
    return out


# revision 23
# speedup vs baseline: 1.0398x; 1.0398x over previous
"""LSTM layer kernel for Trainium2 (8 NeuronCores, Bass/Tile).

Reference computation (fp32):
    z = concat([x, h], axis=1)                 # [B, IN+OUT]
    f = sigmoid(z @ w_f + b_f)
    i = sigmoid(z @ w_i + b_i)
    g = tanh   (z @ w_c + b_c)
    o = sigmoid(z @ w_o + b_o)
    c_new = c * f + i * g
    h_new = tanh(c_new) * o                    # [B, OUT]

Shapes: B=4096, IN=OUT=1024, K=IN+OUT=2048.

Sharding (8 cores): 2-D grid, 4 batch-groups x 2 output-column-groups.
Core (i, j) computes h_new[i*1024:(i+1)*1024, j*512:(j+1)*512].
Per-core HBM traffic (bf16 matmul operands): 4 MiB zT + 8 MiB weights
+ 2 MiB cT + 2 MiB out = 16 MiB.  No collectives.

Device layout (weight-stationary): contraction dim k and out-channel dim
o sit on SBUF partitions:
    zT [2048, 1024] bf16 (k, b)            - moving operand
    w  [4, 2048, 4, 128] bf16 (oc, k, gate, p) - stationary operand,
        oc-major so any ko-slice of one oc is HBM-contiguous
    psum[o, b] = w_chunk.T @ z_chunk; per-o bias is a per-partition ACT
    bias, so sigmoid/tanh drain PSUM directly (ACT reads PSUM fast; a
    DVE drain measures ~10x slower - measured, do not "fix" this).
Host pre-transposes/casts shards, transposes h_newT shards back.

bf16 operands: 1 cycle/row at N=512 (the f32r rate), but bf16 weights
get a split InstLdweights that can be DEDUPED: bass emits one ldweights
per matmul; _dedupe_ldweights removes reloads of the already-loaded
stationary tile (validated bit-exact on HW), so each weight chunk loads
once per oc and serves the two batch-half matmuls.  bf16 quantization
of z and w costs 1.9e-3 rel err vs fp32 reference (gate 2e-2).

Loop structure (per oc): gate-outer, ko-inner, batch-half pairs - gate
g's 16-ko accumulation (32 MMs, ~7us) overlaps the previous gate's ACT
drains, so PSUM banks always have slack (structures where all 8 banks
finish simultaneously stall 10-25us at pass boundaries - measured).
tanh(c*f+i*g) is emitted right after gate c so only the o-gate ACT,
one DVE mul and the out-DMA trail the last matmul.
"""

import numpy as np

import concourse.bass as bass
import concourse.tile as tile
from concourse import bacc
from concourse import mybir
from concourse.bass_utils import run_bass_kernel_spmd

P = 128
B_FULL, IN, OUT = 4096, 1024, 1024
K = IN + OUT                 # 2048 contraction
RB, RO = 4, 2                # batch-shards x out-col-shards = 8 cores
B_L = B_FULL // RB           # 1024 batch rows per core
O_L = OUT // RO              # 512 out cols per core
KO = K // P                  # 16 k-chunks
OC = O_L // P                # 4 out chunks per core
NG = 4                       # gates
NT = 512                     # moving free dim per matmul
NB = B_L // NT               # 2 batch tiles

F32 = mybir.dt.float32
BF16 = mybir.dt.bfloat16
GATES = ("f", "i", "c", "o")

# exec time of the most recent traced run (ns); set by _run when trace=True
last_exec_time_ns = None

# In the loop_r timing variant, emit this many full kernel bodies per
# For_i iteration.  tc.For_i inserts a full cross-engine drain+barrier
# reset block every iteration (~10us) which also kills cross-execution
# DMA/compute overlap; unrolling two bodies per iteration lets body k+1's
# input stream overlap body k's compute (the pools are double-buffered)
# and amortizes the reset, so the slope measures steady-state
# back-to-back execution time.  test.py divides the slope by this.
LOOP_UNROLL = 4

_NC_CACHE = {}


def _dedupe_ldweights(nc):
    """Remove InstLdweights that reload the stationary operand already in
    the PE array (same access pattern as the previous ldweights, nothing
    else touching the PE weight array in between, no sync side effects).
    bass emits one ldweights per matmul; consecutive matmuls sharing lhsT
    only need the first.  Validated bit-exact on TRN2 hardware."""
    removed = 0
    for blk in nc.m.functions[0].blocks:
        prev_key = None
        to_remove = []
        for i in list(blk.instructions):
            tn = type(i).__name__
            if tn == "InstLdweights":
                ap = i.ins[0]
                key = (ap.memref, ap.offset, str(ap.ap))
                si = i.sync_info
                has_sync = si is not None and (
                    len(si.on_wait) > 0 or len(si.on_update) > 0
                )
                if key == prev_key and not has_sync:
                    to_remove.append(i)
                else:
                    prev_key = key
            elif tn in ("InstMatmult", "InstActivation", "InstTensorTensor",
                        "InstTensorScalar", "InstTensorCopy", "InstDMACopy",
                        "InstEventSemaphore", "InstLoadActFuncSet",
                        "InstMemset", "InstIota"):
                pass  # these don't modify the PE weight array
            else:
                prev_key = None
        for i in to_remove:
            blk.instructions.remove(i)
        removed += len(to_remove)
    return removed


def _build_nc(loop_r=None):
    # loop_r: timing-only variant that repeats the whole body in a hardware
    # For_i loop (see test.py hw_loop_slope).
    nc = bacc.Bacc()

    zT = nc.dram_tensor("zT", [K, B_L], BF16, kind="ExternalInput")
    cT = nc.dram_tensor("cT", [O_L, B_L], F32, kind="ExternalInput")
    # oc-major gate-fused weights: [oc, k, gate, p], o_local = oc*128 + p
    wA = nc.dram_tensor("wA", [OC, K, NG, P], BF16, kind="ExternalInput")
    # gate-fused biases: [p, oc, gate]
    bA = nc.dram_tensor("bA", [P, OC, NG], F32, kind="ExternalInput")
    hT = nc.dram_tensor("hT", [O_L, B_L], F32, kind="ExternalOutput")

    zT_t = zT[:, :].rearrange("(ko kp) b -> kp ko b", kp=P)    # [128,16,1024]
    cT_t = cT[:, :].rearrange("(oc p) b -> p oc b", p=P)       # [128,4,1024]
    hT_t = hT[:, :].rearrange("(oc p) b -> p oc b", p=P)
    wA_t = wA[:, :, :, :].rearrange(
        "oc (ko kp) g p -> kp oc ko (g p)", kp=P
    )                                                          # [128,4,16,512]

    sig = mybir.ActivationFunctionType.Sigmoid
    tanh = mybir.ActivationFunctionType.Tanh

    import contextlib

    with tile.TileContext(nc) as tc:
        with (
            tc.For_i(0, loop_r, 1) if loop_r else contextlib.nullcontext(),
            tc.tile_pool(name="zpool", bufs=2) as zpool,
            tc.tile_pool(name="cpool", bufs=2) as cpool,
            tc.tile_pool(name="bpool", bufs=1) as bpool,
            tc.tile_pool(name="wpool", bufs=3) as wpool,
            tc.tile_pool(name="gates", bufs=2) as gpool,
            tc.tile_pool(name="temps", bufs=2) as tpool,
            tc.tile_pool(name="psum", bufs=1, space="PSUM") as psum_pool,
        ):
          for _rep in range(LOOP_UNROLL if loop_r else 1):
            z_sb = zpool.tile([P, KO, B_L], BF16, tag="z", name="z_sb")
            w_tiles = [
                wpool.tile([P, KO, NG * P], BF16, tag="w", name=f"w_oc{oc}")
                for oc in range(OC)
            ]
            c_tiles = [
                cpool.tile([P, B_L], F32, tag="c", name=f"c_oc{oc}")
                for oc in range(OC)
            ]
            b_sb = bpool.tile([P, OC, NG], F32, tag="b", name="b_sb")

            # DMA schedule.  Sync-ring FIFO = arrival order: z and the first
            # oc's weights stream together in 2-ko units (256 KiB each, both
            # HBM-contiguous) so the first gate pass starts ~2us in; later
            # weight/c tiles follow as whole contiguous blocks well before
            # the PE needs them.  Bias + c0 ride the scalar ring.
            nc.scalar.dma_start(b_sb[:, :, :], bA[:, :, :])
            nc.scalar.dma_start(c_tiles[0][:, :], cT_t[:, 0, :])
            KU = 2                                             # ko per unit
            for ku in range(KO // KU):
                ks = slice(ku * KU, (ku + 1) * KU)
                if ku == 0:
                    # 1-ko pieces, nb0 z half + w0 first, so the first
                    # matmul (gate f, nb0) launches off 256 KiB of arrivals
                    # (single-shot startup; amortized away in the loop)
                    k1s = slice(0, 1)
                    nc.sync.dma_start(z_sb[:, k1s, 0:NT], zT_t[:, k1s, 0:NT])
                    nc.sync.dma_start(w_tiles[0][:, k1s, :],
                                      wA_t[:, 0, k1s, :])
                    nc.sync.dma_start(z_sb[:, k1s, NT:B_L],
                                      zT_t[:, k1s, NT:B_L])
                    k1s = slice(1, 2)
                    nc.sync.dma_start(z_sb[:, k1s, :], zT_t[:, k1s, :])
                    nc.sync.dma_start(w_tiles[0][:, k1s, :],
                                      wA_t[:, 0, k1s, :])
                    continue
                nc.sync.dma_start(z_sb[:, ks, :], zT_t[:, ks, :])
                nc.sync.dma_start(w_tiles[0][:, ks, :], wA_t[:, 0, ks, :])
            nc.sync.dma_start(w_tiles[1][:, :, :], wA_t[:, 1, :, :])
            nc.sync.dma_start(c_tiles[1][:, :], cT_t[:, 1, :])
            nc.sync.dma_start(w_tiles[2][:, :, :], wA_t[:, 2, :, :])
            nc.sync.dma_start(c_tiles[2][:, :], cT_t[:, 2, :])
            nc.sync.dma_start(w_tiles[3][:, :, :], wA_t[:, 3, :, :])
            nc.sync.dma_start(c_tiles[3][:, :], cT_t[:, 3, :])

            for oc in range(OC):
                w_sb = w_tiles[oc]
                c_sb = c_tiles[oc]

                gate_sb = {}
                cf_sb = {}
                ps_all = {
                    g: [
                        psum_pool.tile([P, NT], F32, tag=f"ps_{g}{nb}",
                                       name=f"ps_{g}{nb}")
                        for nb in range(NB)
                    ]
                    for g in GATES
                }
                if oc == 0:
                    # ko-outer so the PE consumes each arriving z chunk for
                    # all 8 matmuls (1.7us) - slower than the DMA delivers
                    # (~1.1us) - instead of racing ahead per gate and
                    # stalling ~10us on the z stream (gate-outer consumes a
                    # chunk in 0.43us).  Uses all 8 psum banks; the oc1
                    # boundary stall is only the first ACT drains (~0.5us).
                    for ko in range(KO):
                        for gi, g in enumerate(GATES):
                            for nb in range(NB):
                                nc.tensor.matmul(
                                    ps_all[g][nb][:, :],
                                    lhsT=w_sb[:, ko, gi * P:(gi + 1) * P],
                                    rhs=z_sb[:, ko, nb * NT:(nb + 1) * NT],
                                    start=(ko == 0),
                                    stop=(ko == KO - 1),
                                )
                for gi, g in enumerate(GATES):
                    ps = ps_all[g]
                    if oc != 0:
                        for ko in range(KO):
                            for nb in range(NB):
                                # the two batch halves share the stationary
                                # w chunk; dedupe leaves one ldweights per ko
                                nc.tensor.matmul(
                                    ps[nb][:, :],
                                    lhsT=w_sb[:, ko, gi * P:(gi + 1) * P],
                                    rhs=z_sb[:, ko, nb * NT:(nb + 1) * NT],
                                    start=(ko == 0),
                                    stop=(ko == KO - 1),
                                )
                    func = tanh if g == "c" else sig
                    for nb in range(NB):
                        gt = gpool.tile(
                            [P, NT], F32, tag=f"gate_{g}_{nb}",
                            name=f"gate_{g}_{nb}",
                        )
                        nc.scalar.activation(
                            gt[:, :], ps[nb][:, :], func,
                            bias=b_sb[:, oc, gi:gi + 1],
                        )
                        gate_sb[(g, nb)] = gt

                    if g == "c":
                        # tanh(c*f + i*g) does not depend on gate o - emit it
                        # now so only mul+DMA remain after the last matmul
                        for nb in range(NB):
                            bsl = slice(nb * NT, (nb + 1) * NT)
                            cf = tpool.tile([P, NT], F32, tag=f"cf{nb}",
                                            name=f"cf_{nb}")
                            nc.vector.tensor_mul(
                                cf[:, :], c_sb[:, bsl],
                                gate_sb[("f", nb)][:, :]
                            )
                            ig = tpool.tile([P, NT], F32, tag=f"ig{nb}",
                                            name=f"ig{nb}")
                            nc.vector.tensor_mul(
                                ig[:, :], gate_sb[("i", nb)][:, :],
                                gate_sb[("c", nb)][:, :],
                            )
                            nc.vector.tensor_add(cf[:, :], cf[:, :], ig[:, :])
                            nc.scalar.activation(cf[:, :], cf[:, :], tanh)
                            cf_sb[nb] = cf

                for nb in range(NB):
                    bsl = slice(nb * NT, (nb + 1) * NT)
                    cf = cf_sb[nb]
                    if oc == OC - 1:
                        # half-width pieces pipeline the o-mul with the
                        # out-DMA so less trails the final matmul
                        for hh in range(2):
                            hs = slice(hh * (NT // 2), (hh + 1) * (NT // 2))
                            ob = slice(nb * NT + hh * (NT // 2),
                                       nb * NT + (hh + 1) * (NT // 2))
                            nc.vector.tensor_mul(
                                cf[:, hs], cf[:, hs],
                                gate_sb[("o", nb)][:, hs]
                            )
                            nc.gpsimd.dma_start(hT_t[:, oc, ob], cf[:, hs])
                        continue
                    nc.vector.tensor_mul(
                        cf[:, :], cf[:, :], gate_sb[("o", nb)][:, :]
                    )
                    nc.gpsimd.dma_start(hT_t[:, oc, bsl], cf[:, :])

    _dedupe_ldweights(nc)
    # run the Bacc pass pipeline (alloc_regs, wait-splitting, ...);
    # run_bass_via_pjrt does not finalize on our behalf
    nc.finalize()
    return nc


def _get_nc():
    if "nc" not in _NC_CACHE:
        _NC_CACHE["nc"] = _build_nc()
    return _NC_CACHE["nc"]


def _shard_inputs(x, h, c, w_f, b_f, w_i, b_i, w_c, b_c, w_o, b_o):
    import ml_dtypes

    ws = {"f": w_f, "i": w_i, "c": w_c, "o": w_o}
    bz = {"f": b_f, "i": b_i, "c": b_c, "o": b_o}
    f32 = np.float32
    bf16 = ml_dtypes.bfloat16

    # per-out-group fused weight/bias shards (shared by the 4 batch groups)
    # wA[oc, k, g, p] = w_g[k, j*O_L + oc*P + p]
    wA_sh = {}
    bA_sh = {}
    for j in range(RO):
        cols = slice(j * O_L, (j + 1) * O_L)
        wj = np.stack(
            [np.asarray(ws[g][:, cols], dtype=f32).reshape(K, OC, P)
             for g in GATES],
            axis=2,
        )                                       # [K, OC, NG, P]
        wA_sh[j] = np.ascontiguousarray(
            wj.transpose(1, 0, 2, 3)
        ).astype(bf16)                          # [OC, K, NG, P]
        bA_sh[j] = np.ascontiguousarray(
            np.stack(
                [np.asarray(bz[g], dtype=f32).reshape(-1)[cols].reshape(OC, P).T
                 for g in GATES],
                axis=2,
            )
        )
    in_maps = []
    for i in range(RB):
        rows = slice(i * B_L, (i + 1) * B_L)
        zT = np.ascontiguousarray(
            np.concatenate([x[rows], h[rows]], axis=1).T.astype(bf16)
        )
        for j in range(RO):
            cT = np.ascontiguousarray(
                c[rows, j * O_L:(j + 1) * O_L].T, dtype=f32
            )
            in_maps.append(
                {"zT": zT, "cT": cT, "wA": wA_sh[j], "bA": bA_sh[j]}
            )
    return in_maps


def _run(in_maps, trace=False, trace_cores=None):
    global last_exec_time_ns
    nc = _get_nc()
    res = run_bass_kernel_spmd(
        nc, in_maps, list(range(RB * RO)),
        trace=trace, trace_cores=trace_cores,
    )
    if trace:
        last_exec_time_ns = res.exec_time_ns
    return res.results


def kernel(x, h, c, w_f, b_f, w_i, b_i, w_c, b_c, w_o, b_o):
    in_maps = _shard_inputs(
        x, h, c, w_f, b_f, w_i, b_i, w_c, b_c, w_o, b_o
    )
    results = _run(in_maps)
    out = np.empty((B_FULL, OUT), np.float32)
    for i in range(RB):
        for j in range(RO):
            shard = results[i * RO + j]["hT"]  # [O_L, B_L]
            out[i * B_L:(i + 1) * B_L, j * O_L:(j + 1) * O_L] = shard.T
    return out
